# revision 2
# baseline (speedup 1.0000x reference)
"""AbstractKWTA kernel for 8 Trainium2 NeuronCores.

Model (per batch b, all in f32):
    z = weight @ x[b]                      # [N=1024, T=512], C=2048 contraction
    recurrent scan over T:
        tot  = sum_n s
        f    = (1+a)*s + (RBIAS - tot)
        u    = 0.75*u + (z_t + f)
        v    = 0.9*v + u
        s    = (v >= 1)
        v    = v * (1 - s)
    out[..., t+1] = s_t  (one-step delay, out[..., 0] = 0)

Sharding: data-parallel over batch B=64 -> 8 cores x 8 batches.
Per core: z matmul produced in [t, n] layout (stationary = x tile), staged in
DRAM; scan runs on [128 partitions = (batch, neuron-group), 64 free] tiles.
The per-step cross-partition reduction (sum over neurons) is a tiny f32
matmul against a block matrix of -1s; RBIAS is folded in exactly via a
constant -RBIAS/16 column appended to each spike tile.
"""

import numpy as np

import concourse.bacc as bacc
import concourse.tile as tile
from concourse import mybir
from concourse.bass_utils import run_bass_kernel_spmd
from concourse._compat import get_trn_type

dt = mybir.dt

B, C, T, N = 64, 2048, 512, 1024
NCORES = 8
BL = B // NCORES          # batches per core
RB = np.float32(1.0 * (N - 200) / N)   # 0.8046875, exactly representable
CH = 16                   # scan steps per I/O chunk
KC = C // 128             # 16 contraction tiles

_cache = {}


def _build():
    nc = bacc.Bacc(get_trn_type() or "TRN2", target_bir_lowering=False,
                   debug=False, num_devices=NCORES)
    xs = nc.dram_tensor("xs", [BL, C, T], dt.float32, kind="ExternalInput").ap()
    wt = nc.dram_tensor("wt", [C, N], dt.float32, kind="ExternalInput").ap()
    a1_in = nc.dram_tensor("a1", [128, 1], dt.float32, kind="ExternalInput").ap()
    mneg_in = nc.dram_tensor("mneg", [128, 128], dt.float32, kind="ExternalInput").ap()
    sout = nc.dram_tensor("sout", [T, BL * N], dt.float32, kind="ExternalOutput").ap()

    with tile.TileContext(nc) as tc:
        with tc.tile_pool(name="const", bufs=1) as constp, \
             tc.tile_pool(name="wsb", bufs=1) as wsbp, \
             tc.tile_pool(name="xsb", bufs=2) as xsbp, \
             tc.tile_pool(name="zps", bufs=4, space="PSUM") as zpsp, \
             tc.tile_pool(name="zstg", bufs=4) as zstgp, \
             tc.tile_pool(name="zdram", bufs=1, space="DRAM") as zdramp, \
             tc.tile_pool(name="state", bufs=1) as statep, \
             tc.tile_pool(name="zch", bufs=3) as zchp, \
             tc.tile_pool(name="sch", bufs=3) as schp, \
             tc.tile_pool(name="tmp", bufs=4) as tmpp, \
             tc.tile_pool(name="cps", bufs=2, space="PSUM") as cpsp:

            a1 = constp.tile([128, 1], dt.float32, tag="a1")
            nc.sync.dma_start(a1[:], a1_in[:])
            mneg = constp.tile([128, 128], dt.float32, tag="mneg")
            nc.sync.dma_start(mneg[:], mneg_in[:])

            w_sb = wsbp.tile([128, KC * N], dt.float32, tag="w")
            nc.sync.dma_start(
                w_sb[:].rearrange("p (k n) -> p k n", n=N),
                wt.rearrange("(k p) n -> p k n", p=128))

            # ---- phase 1: z[t, n] = x[b].T @ w.T, staged to DRAM [t, b*n] ----
            zscr = zdramp.tile([T, BL * N], dt.float32, tag="zscr")
            x_re = xs.rearrange("b (k p) t -> b p k t", p=128)
            for tcb in range(T // 128):
                for b in range(BL):
                    x_sb = xsbp.tile([128, KC * 128], dt.float32, tag="x")
                    nc.sync.dma_start(
                        x_sb[:].rearrange("p (k t) -> p k t", t=128),
                        x_re[b, :, :, tcb * 128:(tcb + 1) * 128])
                    for nc2 in range(2):
                        zp = zpsp.tile([128, 512], dt.float32, tag="zp")
                        for k in range(KC):
                            nc.tensor.matmul(
                                zp[:],
                                x_sb[:, k * 128:(k + 1) * 128],
                                w_sb[:, k * N + nc2 * 512:k * N + nc2 * 512 + 512],
                                start=(k == 0), stop=(k == KC - 1))
                        zs = zstgp.tile([128, 512], dt.float32, tag="zs")
                        nc.scalar.copy(zs[:], zp[:])
                        nc.sync.dma_start(
                            zscr[tcb * 128:(tcb + 1) * 128,
                                 b * N + nc2 * 512:b * N + nc2 * 512 + 512],
                            zs[:])

            # ---- phase 2: recurrent scan ----
            u = statep.tile([128, 64], dt.float32, tag="u")
            v = statep.tile([128, 64], dt.float32, tag="v")
            nc.vector.memset(u[:], 0.0)
            nc.vector.memset(v[:], 0.0)
            s_init = statep.tile([128, 65], dt.float32, tag="sinit")
            nc.vector.memset(s_init[:, 0:64], 0.0)
            nc.vector.memset(s_init[:, 64:65], -float(RB) / 16.0)

            zscr_re = zscr.rearrange("t (p e) -> p t e", p=128)
            sout_re = sout.rearrange("t (p e) -> p t e", p=128)
            s_prev, s_off = s_init, 0
            for ci in range(T // CH):
                zch = zchp.tile([128, CH * 64], dt.float32)
                nc.sync.dma_start(
                    zch[:].rearrange("p (k e) -> p k e", e=64),
                    zscr_re[:, ci * CH:(ci + 1) * CH, :])
                sch = schp.tile([128, CH * 65], dt.float32)
                nc.gpsimd.memset(
                    sch[:].rearrange("p (k e) -> p k e", e=65)[:, :, 64:65],
                    -float(RB) / 16.0)
                for k in range(CH):
                    sco = k * 65
                    partial = tmpp.tile([128, 1], dt.float32, tag="partial")
                    nc.vector.tensor_reduce(
                        partial[:], s_prev[:, s_off:s_off + 65],
                        mybir.AxisListType.X, mybir.AluOpType.add)
                    ctot = cpsp.tile([128, 1], dt.float32, tag="ctot")
                    nc.tensor.matmul(ctot[:], mneg[:], partial[:],
                                     start=True, stop=True)
                    f = tmpp.tile([128, 64], dt.float32, tag="f")
                    nc.vector.tensor_scalar(
                        f[:], s_prev[:, s_off:s_off + 64], a1[:], ctot[:],
                        mybir.AluOpType.mult, mybir.AluOpType.add)
                    inp = tmpp.tile([128, 64], dt.float32, tag="inp")
                    nc.vector.tensor_tensor(
                        inp[:], zch[:, k * 64:(k + 1) * 64], f[:],
                        mybir.AluOpType.add)
                    mu = tmpp.tile([128, 64], dt.float32, tag="mu")
                    nc.scalar.mul(mu[:], u[:], 0.75)
                    nc.vector.tensor_tensor(u[:], mu[:], inp[:],
                                            mybir.AluOpType.add)
                    mv = tmpp.tile([128, 64], dt.float32, tag="mv")
                    nc.scalar.mul(mv[:], v[:], 0.9)
                    vpre = tmpp.tile([128, 64], dt.float32, tag="vpre")
                    nc.vector.tensor_tensor(vpre[:], mv[:], u[:],
                                            mybir.AluOpType.add)
                    nc.vector.tensor_scalar(
                        sch[:, sco:sco + 64], vpre[:], 1.0, None,
                        mybir.AluOpType.is_ge)
                    ns = tmpp.tile([128, 64], dt.float32, tag="ns")
                    nc.vector.tensor_scalar(ns[:], vpre[:], 1.0, None,
                                            mybir.AluOpType.is_lt)
                    nc.vector.tensor_tensor(v[:], vpre[:], ns[:],
                                            mybir.AluOpType.mult)
                    s_prev, s_off = sch, sco
                nc.sync.dma_start(
                    sout_re[:, ci * CH:(ci + 1) * CH, :],
                    sch[:].rearrange("p (k e) -> p k e", e=65)[:, :, 0:64])
    nc.compile()
    return nc


def kernel(x, weight, self_excitation):
    x = np.ascontiguousarray(np.asarray(x, dtype=np.float32))
    weight = np.asarray(weight, dtype=np.float32)
    a = np.float32(np.clip(np.asarray(self_excitation, np.float32), 0.0, 1.0)[0])
    A1 = np.float32(np.float32(1.0) + a)

    if "nc" not in _cache:
        _cache["nc"] = _build()
    nc = _cache["nc"]

    wT = np.ascontiguousarray(weight.T)                     # [C, N]
    a1v = np.full((128, 1), A1, np.float32)
    blk = np.repeat(np.arange(8), 16)
    mneg = np.where(blk[:, None] == blk[None, :], np.float32(-1.0),
                    np.float32(0.0)).astype(np.float32)

    in_maps = []
    for c in range(NCORES):
        in_maps.append({
            "xs": x[c * BL:(c + 1) * BL],
            "wt": wT,
            "a1": a1v,
            "mneg": mneg,
        })
    global _last_in_maps
    _last_in_maps = in_maps
    res = run_bass_kernel_spmd(nc, in_maps, core_ids=list(range(NCORES)))

    out = np.zeros((B, N, T), np.float32)
    for c in range(NCORES):
        g = res.results[c]["sout"].reshape(T, BL, N)        # [t, b, n]
        # delay shift: out[..., t+1] = s_t
        out[c * BL:(c + 1) * BL, :, 1:] = g[:T - 1].transpose(1, 2, 0)
    return out


# revision 6
# speedup vs baseline: 1.3473x; 1.3473x over previous
"""AbstractKWTA kernel for 8 Trainium2 NeuronCores.

Model (per batch b, all in f32):
    z = weight @ x[b]                      # [N=1024, T=512], C=2048 contraction
    recurrent scan over T:
        tot  = sum_n s
        u    = 0.75*u + z_t + (1+a)*s + (RBIAS - tot)
        v    = 0.9*v + u
        s    = (v >= 1);  v = v * (1 - s)
    out[..., t+1] = s_t  (one-step delay, out[..., 0] = 0)

Sharding: data-parallel over batch B=64 -> 8 cores x 8 batches.

z matmul: split-fp32r — operands are split hi/lo at 12 significant bits
(fp32r products are exact at that width), z = Xh'Wh + Xh'Wl + Xl'Wh runs at
1 cycle/row vs fp32's 4, with fp32-class accuracy (err ~2e-7). Stationary is
the x tile so z lands in [t, n] layout, staged through DRAM.

Scan: [128 partitions = (batch, neuron-group), 64 free] tiles. Per step the
cross-partition spike total comes from the sge op's accumulate output fed to
a tiny bf16 PE matmul that accumulates -tot onto an RBIAS-prefilled PSUM
column. u is updated as (0.75u + z) + (RBIAS - tot) + (1+a)s, a rounding
order verified to reproduce the reference spikes bit-exactly.
"""

import numpy as np

import concourse.bacc as bacc
import concourse.tile as tile
from concourse import mybir
from concourse.bass_utils import run_bass_kernel_spmd
from concourse._compat import get_trn_type

dt = mybir.dt

B, C, T, N = 64, 2048, 512, 1024
NCORES = 8
BL = B // NCORES          # batches per core
RB = np.float32(1.0 * (N - 200) / N)   # 0.8046875: exact in f32 and bf16
CH = 16                   # scan steps per I/O chunk
KC = C // 128             # 16 contraction tiles

_cache = {}


def _build(a1_val: float):
    nc = bacc.Bacc(get_trn_type() or "TRN2", target_bir_lowering=False,
                   debug=False, num_devices=NCORES)
    xh_in = nc.dram_tensor("xh", [BL, C, T], dt.float32r, kind="ExternalInput").ap()
    xl_in = nc.dram_tensor("xl", [BL, C, T], dt.float32r, kind="ExternalInput").ap()
    wh_in = nc.dram_tensor("wh", [C, N], dt.float32r, kind="ExternalInput").ap()
    wl_in = nc.dram_tensor("wl", [C, N], dt.float32r, kind="ExternalInput").ap()
    mneg_in = nc.dram_tensor("mneg", [128, 128], dt.bfloat16, kind="ExternalInput").ap()
    rbrow_in = nc.dram_tensor("rbrow", [1, 128], dt.bfloat16, kind="ExternalInput").ap()
    sout = nc.dram_tensor("sout", [T, BL * N], dt.float32, kind="ExternalOutput").ap()

    with tile.TileContext(nc) as tc:
        with tc.tile_pool(name="const", bufs=1) as constp, \
             tc.tile_pool(name="wsb", bufs=1) as wsbp, \
             tc.tile_pool(name="xsb", bufs=2) as xsbp, \
             tc.tile_pool(name="zps", bufs=2, space="PSUM") as zpsp, \
             tc.tile_pool(name="zstg", bufs=4) as zstgp, \
             tc.tile_pool(name="zdram", bufs=1, space="DRAM") as zdramp, \
             tc.tile_pool(name="state", bufs=1) as statep, \
             tc.tile_pool(name="zch", bufs=3) as zchp, \
             tc.tile_pool(name="sch", bufs=3) as schp, \
             tc.tile_pool(name="tmp", bufs=4) as tmpp, \
             tc.tile_pool(name="cps", bufs=1, space="PSUM") as cpsp:

            mneg = constp.tile([128, 128], dt.bfloat16, tag="mneg")
            nc.sync.dma_start(mneg[:], mneg_in[:])
            rbrow = constp.tile([1, 128], dt.bfloat16, tag="rbrow")
            nc.sync.dma_start(rbrow[:], rbrow_in[:])
            onec = constp.tile([1, T], dt.bfloat16, tag="onec")
            nc.vector.memset(onec[:], 1.0)
            zero64 = constp.tile([128, 64], dt.float32, tag="zero64")
            nc.vector.memset(zero64[:], 0.0)

            wh_sb = wsbp.tile([128, KC * N], dt.float32r, tag="wh")
            nc.sync.dma_start(
                wh_sb[:].rearrange("p (k n) -> p k n", n=N),
                wh_in.rearrange("(k p) n -> p k n", p=128))
            wl_sb = wsbp.tile([128, KC * N], dt.float32r, tag="wl")
            nc.sync.dma_start(
                wl_sb[:].rearrange("p (k n) -> p k n", n=N),
                wl_in.rearrange("(k p) n -> p k n", p=128))

            # ---- phase 1: z[t, n] via split-fp32r, staged to DRAM [t, b*n] ----
            zscr = zdramp.tile([T, BL * N], dt.float32, tag="zscr")
            xh_re = xh_in.rearrange("b (k p) t -> b p k t", p=128)
            xl_re = xl_in.rearrange("b (k p) t -> b p k t", p=128)
            for tcb in range(T // 128):
                for b in range(BL):
                    xh_sb = xsbp.tile([128, KC * 128], dt.float32r, tag="xh")
                    nc.sync.dma_start(
                        xh_sb[:].rearrange("p (k t) -> p k t", t=128),
                        xh_re[b, :, :, tcb * 128:(tcb + 1) * 128])
                    xl_sb = xsbp.tile([128, KC * 128], dt.float32r, tag="xl")
                    nc.sync.dma_start(
                        xl_sb[:].rearrange("p (k t) -> p k t", t=128),
                        xl_re[b, :, :, tcb * 128:(tcb + 1) * 128])
                    zp0 = zpsp.tile([128, 512], dt.float32, tag="zp0")
                    zp1 = zpsp.tile([128, 512], dt.float32, tag="zp1")
                    zp = (zp0, zp1)
                    for k in range(KC):
                        xh_k = xh_sb[:, k * 128:(k + 1) * 128]
                        xl_k = xl_sb[:, k * 128:(k + 1) * 128]
                        for nc2 in range(2):
                            wh_k = wh_sb[:, k * N + nc2 * 512:k * N + nc2 * 512 + 512]
                            wl_k = wl_sb[:, k * N + nc2 * 512:k * N + nc2 * 512 + 512]
                            nc.tensor.matmul(zp[nc2][:], xh_k, wh_k,
                                             start=(k == 0), stop=False,
                                             skip_group_check=True)
                            nc.tensor.matmul(zp[nc2][:], xh_k, wl_k,
                                             start=False, stop=False,
                                             skip_group_check=True)
                            nc.tensor.matmul(zp[nc2][:], xl_k, wh_k,
                                             start=False, stop=(k == KC - 1),
                                             skip_group_check=True)
                    for nc2 in range(2):
                        zs = zstgp.tile([128, 512], dt.float32, tag="zs")
                        nc.scalar.copy(zs[:], zp[nc2][:])
                        nc.sync.dma_start(
                            zscr[tcb * 128:(tcb + 1) * 128,
                                 b * N + nc2 * 512:b * N + nc2 * 512 + 512],
                            zs[:])

            # ---- phase 2: recurrent scan ----
            u = statep.tile([128, 64], dt.float32, tag="u")
            v = statep.tile([128, 64], dt.float32, tag="v")
            nc.vector.memset(u[:], 0.0)
            nc.vector.memset(v[:], 0.0)
            s_init = statep.tile([128, 64], dt.float32, tag="sinit")
            nc.vector.memset(s_init[:], 0.0)
            part_init = statep.tile([128, 1], dt.bfloat16, tag="pinit")
            nc.vector.memset(part_init[:], 0.0)

            c_all = cpsp.tile([128, T], dt.float32, tag="call")
            nc.tensor.matmul(c_all[:], rbrow[:], onec[:], start=True, stop=False,
                             skip_group_check=True)

            zscr_re = zscr.rearrange("t (p e) -> p t e", p=128)
            sout_re = sout.rearrange("t (p e) -> p t e", p=128)
            s_prev, partial = s_init, part_init
            for ci in range(T // CH):
                zch = zchp.tile([128, CH * 64], dt.float32)
                nc.sync.dma_start(
                    zch[:].rearrange("p (k e) -> p k e", e=64),
                    zscr_re[:, ci * CH:(ci + 1) * CH, :])
                sch = schp.tile([128, CH * 64], dt.float32)
                for k in range(CH):
                    t = ci * CH + k
                    nc.tensor.matmul(c_all[:, t:t + 1], mneg[:], partial[:],
                                     start=False, stop=(t == T - 1),
                                     skip_group_check=True)
                    mu = tmpp.tile([128, 64], dt.float32, tag="mu")
                    nc.scalar.mul(mu[:], u[:], 0.75)
                    q = tmpp.tile([128, 64], dt.float32, tag="q")
                    nc.gpsimd.tensor_tensor(q[:], mu[:], zch[:, k * 64:(k + 1) * 64],
                                            mybir.AluOpType.add)
                    sA = tmpp.tile([128, 64], dt.float32, tag="sA")
                    nc.gpsimd.tensor_scalar(sA[:], s_prev[:], float(a1_val), None,
                                            mybir.AluOpType.mult)
                    nc.vector.scalar_tensor_tensor(
                        u[:], q[:], c_all[:, t:t + 1], sA[:],
                        mybir.AluOpType.add, mybir.AluOpType.add)
                    mv = tmpp.tile([128, 64], dt.float32, tag="mv")
                    nc.scalar.mul(mv[:], v[:], 0.9)
                    nc.vector.tensor_tensor(v[:], mv[:], u[:], mybir.AluOpType.add)
                    partial = tmpp.tile([128, 1], dt.bfloat16, tag="partial")
                    s_sl = sch[:, k * 64:(k + 1) * 64]
                    nc.vector.tensor_scalar(s_sl, v[:], 1.0, 0.0,
                                            mybir.AluOpType.is_ge,
                                            mybir.AluOpType.add,
                                            accum_out=partial[:])
                    nc.vector.copy_predicated(v[:], s_sl.bitcast(dt.int32),
                                              zero64[:])
                    s_prev = s_sl
                nc.sync.dma_start(
                    sout_re[:, ci * CH:(ci + 1) * CH, :],
                    sch[:].rearrange("p (k e) -> p k e", e=64))
    nc.compile()
    return nc


def _round_mant(a, keep=11):
    """Round f32 to `keep` explicit mantissa bits (round-to-nearest)."""
    bits = a.view(np.uint32)
    shift = 23 - keep
    half = np.uint32(1 << (shift - 1))
    mask = np.uint32((0xFFFFFFFF << shift) & 0xFFFFFFFF)
    return ((bits + half) & mask).view(np.float32)


def _split(a):
    hi = _round_mant(np.ascontiguousarray(a, np.float32))
    lo = _round_mant((a.astype(np.float64) - hi).astype(np.float32))
    return hi, lo


def kernel(x, weight, self_excitation):
    import ml_dtypes
    x = np.asarray(x, dtype=np.float32)
    weight = np.asarray(weight, dtype=np.float32)
    a = np.float32(np.clip(np.asarray(self_excitation, np.float32), 0.0, 1.0)[0])
    A1 = np.float32(np.float32(1.0) + a)

    key = float(A1)
    if _cache.get("key") != key:
        _cache["nc"] = _build(key)
        _cache["key"] = key
    nc = _cache["nc"]

    wT = np.ascontiguousarray(weight.T)                     # [C, N]
    wh, wl = _split(wT)
    xh, xl = _split(x)
    blk = np.repeat(np.arange(8), 16)
    mneg = np.where(blk[:, None] == blk[None, :], np.float32(-1.0),
                    np.float32(0.0)).astype(ml_dtypes.bfloat16)
    rbrow = np.full((1, 128), RB, ml_dtypes.bfloat16)

    in_maps = []
    for c in range(NCORES):
        in_maps.append({
            "xh": xh[c * BL:(c + 1) * BL],
            "xl": xl[c * BL:(c + 1) * BL],
            "wh": wh,
            "wl": wl,
            "mneg": mneg,
            "rbrow": rbrow,
        })
    global _last_in_maps
    _last_in_maps = in_maps
    res = run_bass_kernel_spmd(nc, in_maps, core_ids=list(range(NCORES)))

    out = np.zeros((B, N, T), np.float32)
    for c in range(NCORES):
        g = res.results[c]["sout"].reshape(T, BL, N)        # [t, b, n]
        out[c * BL:(c + 1) * BL, :, 1:] = g[:T - 1].transpose(1, 2, 0)
    return out


# revision 7
# speedup vs baseline: 1.4456x; 1.0730x over previous
"""AbstractKWTA kernel for 8 Trainium2 NeuronCores.

Model (per batch b, all in f32):
    z = weight @ x[b]                      # [N=1024, T=512], C=2048 contraction
    recurrent scan over T:
        tot  = sum_n s
        u    = 0.75*u + z_t + (1+a)*s + (RBIAS - tot)
        v    = 0.9*v + u
        s    = (v >= 1);  v = v * (1 - s)
    out[..., t+1] = s_t  (one-step delay, out[..., 0] = 0)

Sharding: data-parallel over batch B=64 -> 8 cores x 8 batches.

z matmul: split-fp32r — operands are split hi/lo at 12 significant bits
(fp32r products are exact at that width), z = Xh'Wh + Xh'Wl + Xl'Wh runs at
1 cycle/row vs fp32's 4, with fp32-class accuracy (err ~2e-7). Stationary is
the x tile so z lands in [t, n] layout, staged through DRAM.

Scan: [128 partitions = (batch, neuron-group), 64 free] tiles. Per step the
cross-partition spike total comes from the sge op's accumulate output fed to
a tiny bf16 PE matmul that accumulates -tot onto an RBIAS-prefilled PSUM
column. u is updated as (0.75u + z) + (RBIAS - tot) + (1+a)s, a rounding
order verified to reproduce the reference spikes bit-exactly.
"""

import numpy as np

import concourse.bacc as bacc
import concourse.tile as tile
from concourse import mybir
from concourse.bass_utils import run_bass_kernel_spmd
from concourse._compat import get_trn_type

dt = mybir.dt

B, C, T, N = 64, 2048, 512, 1024
NCORES = 8
BL = B // NCORES          # batches per core
RB = np.float32(1.0 * (N - 200) / N)   # 0.8046875: exact in f32 and bf16
CH = 16                   # scan steps per I/O chunk
KC = C // 128             # 16 contraction tiles

_cache = {}


def _build(a1_val: float):
    nc = bacc.Bacc(get_trn_type() or "TRN2", target_bir_lowering=False,
                   debug=False, num_devices=NCORES)
    xh_in = nc.dram_tensor("xh", [BL, C, T], dt.float32r, kind="ExternalInput").ap()
    xl_in = nc.dram_tensor("xl", [BL, C, T], dt.float32r, kind="ExternalInput").ap()
    wh_in = nc.dram_tensor("wh", [C, N], dt.float32r, kind="ExternalInput").ap()
    wl_in = nc.dram_tensor("wl", [C, N], dt.float32r, kind="ExternalInput").ap()
    mneg_in = nc.dram_tensor("mneg", [128, 128], dt.bfloat16, kind="ExternalInput").ap()
    rbrow_in = nc.dram_tensor("rbrow", [1, 128], dt.bfloat16, kind="ExternalInput").ap()
    sout = nc.dram_tensor("sout", [T, BL * N], dt.float32, kind="ExternalOutput").ap()

    with tile.TileContext(nc) as tc:
        with tc.tile_pool(name="const", bufs=1) as constp, \
             tc.tile_pool(name="wsb", bufs=1) as wsbp, \
             tc.tile_pool(name="xsb", bufs=2) as xsbp, \
             tc.tile_pool(name="zps", bufs=2, space="PSUM") as zpsp, \
             tc.tile_pool(name="zstg", bufs=4) as zstgp, \
             tc.tile_pool(name="zdram", bufs=1, space="DRAM") as zdramp, \
             tc.tile_pool(name="state", bufs=1) as statep, \
             tc.tile_pool(name="zch", bufs=3) as zchp, \
             tc.tile_pool(name="sch", bufs=3) as schp, \
             tc.tile_pool(name="tmp", bufs=4) as tmpp, \
             tc.tile_pool(name="cps", bufs=1, space="PSUM") as cpsp:

            mneg = constp.tile([128, 128], dt.bfloat16, tag="mneg")
            nc.sync.dma_start(mneg[:], mneg_in[:])
            rbrow = constp.tile([1, 128], dt.bfloat16, tag="rbrow")
            nc.sync.dma_start(rbrow[:], rbrow_in[:])
            onec = constp.tile([1, T], dt.bfloat16, tag="onec")
            nc.vector.memset(onec[:], 1.0)
            zero64 = constp.tile([128, 64], dt.float32, tag="zero64")
            nc.vector.memset(zero64[:], 0.0)

            wh_sb = wsbp.tile([128, KC * N], dt.float32r, tag="wh")
            nc.sync.dma_start(
                wh_sb[:].rearrange("p (k n) -> p k n", n=N),
                wh_in.rearrange("(k p) n -> p k n", p=128))
            wl_sb = wsbp.tile([128, KC * N], dt.float32r, tag="wl")
            nc.sync.dma_start(
                wl_sb[:].rearrange("p (k n) -> p k n", n=N),
                wl_in.rearrange("(k p) n -> p k n", p=128))

            # ---- phase 1: z[t, n] via split-fp32r, staged to DRAM [t, b*n] ----
            zscr = zdramp.tile([T, BL * N], dt.float32, tag="zscr")
            xh_re = xh_in.rearrange("b (k p) t -> b p k t", p=128)
            xl_re = xl_in.rearrange("b (k p) t -> b p k t", p=128)
            for tcb in range(T // 128):
                for b in range(BL):
                    xh_sb = xsbp.tile([128, KC * 128], dt.float32r, tag="xh")
                    nc.sync.dma_start(
                        xh_sb[:].rearrange("p (k t) -> p k t", t=128),
                        xh_re[b, :, :, tcb * 128:(tcb + 1) * 128])
                    xl_sb = xsbp.tile([128, KC * 128], dt.float32r, tag="xl")
                    nc.sync.dma_start(
                        xl_sb[:].rearrange("p (k t) -> p k t", t=128),
                        xl_re[b, :, :, tcb * 128:(tcb + 1) * 128])
                    zp0 = zpsp.tile([128, 512], dt.float32, tag="zp0")
                    zp1 = zpsp.tile([128, 512], dt.float32, tag="zp1")
                    zp = (zp0, zp1)
                    for k in range(KC):
                        xh_k = xh_sb[:, k * 128:(k + 1) * 128]
                        xl_k = xl_sb[:, k * 128:(k + 1) * 128]
                        for nc2 in range(2):
                            wh_k = wh_sb[:, k * N + nc2 * 512:k * N + nc2 * 512 + 512]
                            wl_k = wl_sb[:, k * N + nc2 * 512:k * N + nc2 * 512 + 512]
                            nc.tensor.matmul(zp[nc2][:], xh_k, wh_k,
                                             start=(k == 0), stop=False,
                                             skip_group_check=True)
                            nc.tensor.matmul(zp[nc2][:], xh_k, wl_k,
                                             start=False, stop=False,
                                             skip_group_check=True)
                            nc.tensor.matmul(zp[nc2][:], xl_k, wh_k,
                                             start=False, stop=(k == KC - 1),
                                             skip_group_check=True)
                    for nc2 in range(2):
                        zs = zstgp.tile([128, 512], dt.float32, tag="zs")
                        nc.scalar.copy(zs[:], zp[nc2][:])
                        nc.sync.dma_start(
                            zscr[tcb * 128:(tcb + 1) * 128,
                                 b * N + nc2 * 512:b * N + nc2 * 512 + 512],
                            zs[:])

            # ---- phase 2: recurrent scan ----
            u = statep.tile([128, 64], dt.float32, tag="u")
            v = statep.tile([128, 64], dt.float32, tag="v")
            nc.vector.memset(u[:], 0.0)
            nc.vector.memset(v[:], 0.0)
            s_init = statep.tile([128, 64], dt.float32, tag="sinit")
            nc.vector.memset(s_init[:], 0.0)
            part_init = statep.tile([128, 1], dt.bfloat16, tag="pinit")
            nc.vector.memset(part_init[:], 0.0)

            c_all = cpsp.tile([128, T], dt.float32, tag="call")
            nc.tensor.matmul(c_all[:], rbrow[:], onec[:], start=True, stop=False,
                             skip_group_check=True)

            zscr_re = zscr.rearrange("t (p e) -> p t e", p=128)
            sout_re = sout.rearrange("t (p e) -> p t e", p=128)
            s_prev, partial = s_init, part_init
            for ci in range(T // CH):
                zch = zchp.tile([128, CH * 64], dt.float32)
                nc.sync.dma_start(
                    zch[:].rearrange("p (k e) -> p k e", e=64),
                    zscr_re[:, ci * CH:(ci + 1) * CH, :])
                sch = schp.tile([128, CH * 64], dt.float32)
                for k in range(CH):
                    t = ci * CH + k
                    nc.tensor.matmul(c_all[:, t:t + 1], mneg[:], partial[:],
                                     start=False, stop=(t == T - 1),
                                     skip_group_check=True)
                    mu = tmpp.tile([128, 64], dt.float32, tag="mu")
                    nc.scalar.mul(mu[:], u[:], 0.75)
                    q = tmpp.tile([128, 64], dt.float32, tag="q")
                    nc.gpsimd.tensor_tensor(q[:], mu[:], zch[:, k * 64:(k + 1) * 64],
                                            mybir.AluOpType.add)
                    sA = tmpp.tile([128, 64], dt.float32, tag="sA")
                    nc.scalar.mul(sA[:], s_prev[:], float(a1_val))
                    nc.vector.scalar_tensor_tensor(
                        u[:], q[:], c_all[:, t:t + 1], sA[:],
                        mybir.AluOpType.add, mybir.AluOpType.add)
                    mv = tmpp.tile([128, 64], dt.float32, tag="mv")
                    nc.scalar.mul(mv[:], v[:], 0.9)
                    nc.vector.tensor_tensor(v[:], mv[:], u[:], mybir.AluOpType.add)
                    partial = tmpp.tile([128, 1], dt.bfloat16, tag="partial")
                    s_sl = sch[:, k * 64:(k + 1) * 64]
                    nc.vector.tensor_scalar(s_sl, v[:], 1.0, 0.0,
                                            mybir.AluOpType.is_ge,
                                            mybir.AluOpType.add,
                                            accum_out=partial[:])
                    nc.vector.copy_predicated(v[:], s_sl.bitcast(dt.int32),
                                              zero64[:])
                    s_prev = s_sl
                nc.sync.dma_start(
                    sout_re[:, ci * CH:(ci + 1) * CH, :],
                    sch[:].rearrange("p (k e) -> p k e", e=64))
    nc.compile()
    return nc


def _round_mant(a, keep=11):
    """Round f32 to `keep` explicit mantissa bits (round-to-nearest)."""
    bits = a.view(np.uint32)
    shift = 23 - keep
    half = np.uint32(1 << (shift - 1))
    mask = np.uint32((0xFFFFFFFF << shift) & 0xFFFFFFFF)
    return ((bits + half) & mask).view(np.float32)


def _split(a):
    hi = _round_mant(np.ascontiguousarray(a, np.float32))
    lo = _round_mant((a.astype(np.float64) - hi).astype(np.float32))
    return hi, lo


def kernel(x, weight, self_excitation):
    import ml_dtypes
    x = np.asarray(x, dtype=np.float32)
    weight = np.asarray(weight, dtype=np.float32)
    a = np.float32(np.clip(np.asarray(self_excitation, np.float32), 0.0, 1.0)[0])
    A1 = np.float32(np.float32(1.0) + a)

    key = float(A1)
    if _cache.get("key") != key:
        _cache["nc"] = _build(key)
        _cache["key"] = key
    nc = _cache["nc"]

    wT = np.ascontiguousarray(weight.T)                     # [C, N]
    wh, wl = _split(wT)
    xh, xl = _split(x)
    blk = np.repeat(np.arange(8), 16)
    mneg = np.where(blk[:, None] == blk[None, :], np.float32(-1.0),
                    np.float32(0.0)).astype(ml_dtypes.bfloat16)
    rbrow = np.full((1, 128), RB, ml_dtypes.bfloat16)

    in_maps = []
    for c in range(NCORES):
        in_maps.append({
            "xh": xh[c * BL:(c + 1) * BL],
            "xl": xl[c * BL:(c + 1) * BL],
            "wh": wh,
            "wl": wl,
            "mneg": mneg,
            "rbrow": rbrow,
        })
    global _last_in_maps
    _last_in_maps = in_maps
    res = run_bass_kernel_spmd(nc, in_maps, core_ids=list(range(NCORES)))

    out = np.zeros((B, N, T), np.float32)
    for c in range(NCORES):
        g = res.results[c]["sout"].reshape(T, BL, N)        # [t, b, n]
        out[c * BL:(c + 1) * BL, :, 1:] = g[:T - 1].transpose(1, 2, 0)
    return out


# revision 11
# speedup vs baseline: 2.2029x; 1.5239x over previous
"""AbstractKWTA kernel for 8 Trainium2 NeuronCores.

Model (per batch b, all in f32):
    z = weight @ x[b]                      # [N=1024, T=512], C=2048 contraction
    recurrent scan over T:
        tot  = sum_n s
        u    = 0.75*u + z_t + (1+a)*s + (RBIAS - tot)
        v    = 0.9*v + u
        s    = (v >= 1);  v = v * (1 - s)
    out[..., t+1] = s_t  (one-step delay, out[..., 0] = 0)

Sharding: data-parallel over batch B=64 -> 8 cores x 8 batches.

z matmul: split-fp32r — operands split hi/lo at 12 significant bits (fp32r
products are exact at that width); z = Xh'Wh + Xh'Wl + Xl'Wh at ~1.3 cyc/row
vs fp32's 4, fp32-class accuracy (err ~2e-7). Stationary is the x tile so z
lands in [t, n] layout, staged through DRAM.

Scan: [128 partitions = (batch, neuron-group), 64 free] tiles, one step every
~1.6us. The per-step critical cycle is kept entirely on the Vector engine:
cpred(reset mv) -> u(STT) -> v(TT) -> sge(+accumulate). The spike total
reaches u via a tiny bf16 PE matmul accumulating -tot onto an RBIAS-prefilled
PSUM column; (1+a)*s comes from ScalarE into PSUM. The voltage reset is
applied to the *decayed* copy mv = 0.9*v (exact: v*(1-s) then decay equals
decay then zero-where-spiked), keeping it off the critical path.

The z-production matmuls for t-blocks 1..3 are interleaved 6-per-step into
the scan of the preceding block, filling the PE's idle time, so only block 0
is produced up front. Rounding orders are chosen so the spike decisions
reproduce the f32 reference bit-exactly (verified: 0/33.5M mismatches).
"""

import numpy as np

import concourse.bacc as bacc
import concourse.tile as tile
from concourse import mybir
from concourse.bass_utils import run_bass_kernel_spmd
from concourse._compat import get_trn_type

dt = mybir.dt

B, C, T, N = 64, 2048, 512, 1024
NCORES = 8
BL = B // NCORES          # batches per core
RB = np.float32(1.0 * (N - 200) / N)   # 0.8046875: exact in f32 and bf16
CH = 16                   # scan steps per chunk; also matmul k-tiles per chunk
KC = C // 128             # 16 contraction tiles
NTCB = T // 128           # 4 t-blocks

_cache = {}


def _build(a1_val: float):
    nc = bacc.Bacc(get_trn_type() or "TRN2", target_bir_lowering=False,
                   debug=False, num_devices=NCORES)
    xh_in = nc.dram_tensor("xh", [BL, C, T], dt.float32r, kind="ExternalInput").ap()
    xl_in = nc.dram_tensor("xl", [BL, C, T], dt.float32r, kind="ExternalInput").ap()
    wh_in = nc.dram_tensor("wh", [C, N], dt.float32r, kind="ExternalInput").ap()
    wl_in = nc.dram_tensor("wl", [C, N], dt.float32r, kind="ExternalInput").ap()
    mneg_in = nc.dram_tensor("mneg", [128, 128], dt.bfloat16, kind="ExternalInput").ap()
    rbrow_in = nc.dram_tensor("rbrow", [1, 128], dt.bfloat16, kind="ExternalInput").ap()
    sout = nc.dram_tensor("sout", [T, BL * N], dt.float32, kind="ExternalOutput").ap()

    with tile.TileContext(nc) as tc:
        with tc.tile_pool(name="const", bufs=1) as constp, \
             tc.tile_pool(name="wsb", bufs=1) as wsbp, \
             tc.tile_pool(name="xsb", bufs=2) as xsbp, \
             tc.tile_pool(name="zps", bufs=2, space="PSUM") as zpsp, \
             tc.tile_pool(name="zstg", bufs=4) as zstgp, \
             tc.tile_pool(name="zdram", bufs=1, space="DRAM") as zdramp, \
             tc.tile_pool(name="state", bufs=1) as statep, \
             tc.tile_pool(name="zch", bufs=3) as zchp, \
             tc.tile_pool(name="sch", bufs=3) as schp, \
             tc.tile_pool(name="tmp", bufs=4) as tmpp, \
             tc.tile_pool(name="cps", bufs=1, space="PSUM") as cpsp, \
             tc.tile_pool(name="aps", bufs=2, space="PSUM") as apsp:

            mneg = constp.tile([128, 128], dt.bfloat16, tag="mneg")
            nc.sync.dma_start(mneg[:], mneg_in[:])
            rbrow = constp.tile([1, 128], dt.bfloat16, tag="rbrow")
            nc.sync.dma_start(rbrow[:], rbrow_in[:])
            onec = constp.tile([1, T], dt.bfloat16, tag="onec")
            nc.vector.memset(onec[:], 1.0)
            zero64 = constp.tile([128, 64], dt.float32, tag="zero64")
            nc.vector.memset(zero64[:], 0.0)

            wh_sb = wsbp.tile([128, KC * N], dt.float32r, tag="wh")
            nc.sync.dma_start(
                wh_sb[:].rearrange("p (k n) -> p k n", n=N),
                wh_in.rearrange("(k p) n -> p k n", p=128))
            wl_sb = wsbp.tile([128, KC * N], dt.float32r, tag="wl")
            nc.sync.dma_start(
                wl_sb[:].rearrange("p (k n) -> p k n", n=N),
                wl_in.rearrange("(k p) n -> p k n", p=128))

            zscr = zdramp.tile([T, BL * N], dt.float32, tag="zscr")
            xh_re = xh_in.rearrange("b (k p) t -> b p k t", p=128)
            xl_re = xl_in.rearrange("b (k p) t -> b p k t", p=128)

            def z_loads(tcb, b):
                xh_sb = xsbp.tile([128, KC * 128], dt.float32r, tag="xh")
                nc.sync.dma_start(
                    xh_sb[:].rearrange("p (k t) -> p k t", t=128),
                    xh_re[b, :, :, tcb * 128:(tcb + 1) * 128])
                xl_sb = xsbp.tile([128, KC * 128], dt.float32r, tag="xl")
                nc.sync.dma_start(
                    xl_sb[:].rearrange("p (k t) -> p k t", t=128),
                    xl_re[b, :, :, tcb * 128:(tcb + 1) * 128])
                zp0 = zpsp.tile([128, 512], dt.float32, tag="zp0")
                zp1 = zpsp.tile([128, 512], dt.float32, tag="zp1")
                return xh_sb, xl_sb, (zp0, zp1)

            def z_mms(g, k):
                xh_sb, xl_sb, zp = g
                xh_k = xh_sb[:, k * 128:(k + 1) * 128]
                xl_k = xl_sb[:, k * 128:(k + 1) * 128]
                for nc2 in range(2):
                    wh_k = wh_sb[:, k * N + nc2 * 512:k * N + nc2 * 512 + 512]
                    wl_k = wl_sb[:, k * N + nc2 * 512:k * N + nc2 * 512 + 512]
                    nc.tensor.matmul(zp[nc2][:], xh_k, wh_k,
                                     start=(k == 0), stop=False,
                                     skip_group_check=True)
                    nc.tensor.matmul(zp[nc2][:], xh_k, wl_k,
                                     start=False, stop=False,
                                     skip_group_check=True)
                    nc.tensor.matmul(zp[nc2][:], xl_k, wh_k,
                                     start=False, stop=(k == KC - 1),
                                     skip_group_check=True)

            def z_stores(tcb, b, g):
                _, _, zp = g
                for nc2 in range(2):
                    zs = zstgp.tile([128, 512], dt.float32, tag="zs")
                    nc.scalar.copy(zs[:], zp[nc2][:])
                    nc.sync.dma_start(
                        zscr[tcb * 128:(tcb + 1) * 128,
                             b * N + nc2 * 512:b * N + nc2 * 512 + 512],
                        zs[:])

            # ---- t-block 0 produced up front ----
            for b in range(BL):
                g = z_loads(0, b)
                for k in range(KC):
                    z_mms(g, k)
                z_stores(0, b, g)

            # remaining groups, one per scan chunk
            zwork = [(tcb, b) for tcb in range(1, NTCB) for b in range(BL)]

            # ---- scan state ----
            u = statep.tile([128, 64], dt.float32, tag="u")
            nc.vector.memset(u[:], 0.0)
            s_prev = statep.tile([128, 64], dt.float32, tag="sinit")
            nc.vector.memset(s_prev[:], 0.0)
            partial = statep.tile([128, 1], dt.bfloat16, tag="pinit")
            nc.vector.memset(partial[:], 0.0)
            mvp = statep.tile([128, 64], dt.float32, tag="mvinit")
            nc.vector.memset(mvp[:], 0.0)
            mu = tmpp.tile([128, 64], dt.float32, tag="mu")
            nc.scalar.mul(mu[:], u[:], 0.75)

            c_all = cpsp.tile([128, T], dt.float32, tag="call")
            nc.tensor.matmul(c_all[:], rbrow[:], onec[:], start=True, stop=False,
                             skip_group_check=True)

            zscr_re = zscr.rearrange("t (p e) -> p t e", p=128)
            sout_re = sout.rearrange("t (p e) -> p t e", p=128)
            zg_handles = {}
            if zwork:
                zg_handles[0] = z_loads(*zwork[0])
            for ci in range(T // CH):
                if ci + 1 < len(zwork):
                    zg_handles[ci + 1] = z_loads(*zwork[ci + 1])
                zg_cur = zg_handles.pop(ci, None)

                zch = zchp.tile([128, CH * 64], dt.float32)
                nc.sync.dma_start(
                    zch[:].rearrange("p (k e) -> p k e", e=64),
                    zscr_re[:, ci * CH:(ci + 1) * CH, :])
                sch = schp.tile([128, CH * 64], dt.float32)
                for k in range(CH):
                    t = ci * CH + k
                    # PE: scan matmul first, then this step's share of z MMs
                    nc.tensor.matmul(c_all[:, t:t + 1], mneg[:], partial[:],
                                     start=False, stop=(t == T - 1),
                                     skip_group_check=True)
                    if zg_cur is not None:
                        z_mms(zg_cur, k)
                    # ACT: (1+a)*s -> PSUM (feeds this step's STT)
                    sA = apsp.tile([128, 64], dt.float32, tag="sA")
                    nc.scalar.mul(sA[:], s_prev[:], float(a1_val))
                    # DVE: zero the decayed voltage where spiked (prev step)
                    nc.vector.copy_predicated(mvp[:], s_prev[:].bitcast(dt.int32),
                                              zero64[:])
                    # GPSIMD: q = mu + z_t
                    q = tmpp.tile([128, 64], dt.float32, tag="q")
                    nc.gpsimd.tensor_tensor(q[:], mu[:], zch[:, k * 64:(k + 1) * 64],
                                            mybir.AluOpType.add)
                    # DVE: u = (q + c) + sA
                    nc.vector.scalar_tensor_tensor(
                        u[:], q[:], c_all[:, t:t + 1], sA[:],
                        mybir.AluOpType.add, mybir.AluOpType.add)
                    # ACT: mu for next step
                    mu = tmpp.tile([128, 64], dt.float32, tag="mu")
                    nc.scalar.mul(mu[:], u[:], 0.75)
                    # DVE: v = mv + u   (mv = zeroed 0.9*v_prev)
                    vp = tmpp.tile([128, 64], dt.float32, tag="vp")
                    nc.vector.tensor_tensor(vp[:], mvp[:], u[:], mybir.AluOpType.add)
                    # ACT: decayed voltage for next step (reset applied next step)
                    mvp = tmpp.tile([128, 64], dt.float32, tag="mvp")
                    nc.scalar.mul(mvp[:], vp[:], 0.9)
                    # DVE: spikes + per-partition count
                    partial = tmpp.tile([128, 1], dt.bfloat16, tag="partial")
                    s_sl = sch[:, k * 64:(k + 1) * 64]
                    nc.vector.tensor_scalar(s_sl, vp[:], 1.0, 0.0,
                                            mybir.AluOpType.is_ge,
                                            mybir.AluOpType.add,
                                            accum_out=partial[:])
                    s_prev = s_sl
                if zg_cur is not None:
                    z_stores(*zwork[ci], zg_cur)
                nc.sync.dma_start(
                    sout_re[:, ci * CH:(ci + 1) * CH, :],
                    sch[:].rearrange("p (k e) -> p k e", e=64))
    nc.compile()
    return nc


def _round_mant(a, keep=11):
    """Round f32 to `keep` explicit mantissa bits (round-to-nearest)."""
    bits = a.view(np.uint32)
    shift = 23 - keep
    half = np.uint32(1 << (shift - 1))
    mask = np.uint32((0xFFFFFFFF << shift) & 0xFFFFFFFF)
    return ((bits + half) & mask).view(np.float32)


def _split(a):
    hi = _round_mant(np.ascontiguousarray(a, np.float32))
    lo = _round_mant((a.astype(np.float64) - hi).astype(np.float32))
    return hi, lo


def kernel(x, weight, self_excitation):
    import ml_dtypes
    x = np.asarray(x, dtype=np.float32)
    weight = np.asarray(weight, dtype=np.float32)
    a = np.float32(np.clip(np.asarray(self_excitation, np.float32), 0.0, 1.0)[0])
    A1 = np.float32(np.float32(1.0) + a)

    key = float(A1)
    if _cache.get("key") != key:
        _cache["nc"] = _build(key)
        _cache["key"] = key
    nc = _cache["nc"]

    wT = np.ascontiguousarray(weight.T)                     # [C, N]
    wh, wl = _split(wT)
    xh, xl = _split(x)
    blk = np.repeat(np.arange(8), 16)
    mneg = np.where(blk[:, None] == blk[None, :], np.float32(-1.0),
                    np.float32(0.0)).astype(ml_dtypes.bfloat16)
    rbrow = np.full((1, 128), RB, ml_dtypes.bfloat16)

    in_maps = []
    for c in range(NCORES):
        in_maps.append({
            "xh": xh[c * BL:(c + 1) * BL],
            "xl": xl[c * BL:(c + 1) * BL],
            "wh": wh,
            "wl": wl,
            "mneg": mneg,
            "rbrow": rbrow,
        })
    global _last_in_maps
    _last_in_maps = in_maps
    res = run_bass_kernel_spmd(nc, in_maps, core_ids=list(range(NCORES)))

    out = np.zeros((B, N, T), np.float32)
    for c in range(NCORES):
        g = res.results[c]["sout"].reshape(T, BL, N)        # [t, b, n]
        out[c * BL:(c + 1) * BL, :, 1:] = g[:T - 1].transpose(1, 2, 0)
    return out
